# revision 21
# baseline (speedup 1.0000x reference)
"""Trainium2 Bass kernel for MindSpeed TE grouped linear (MoE grouped GEMM).

Computes, for E=64 experts with row splits m_splits (sum = 32768):
    y[rows_e, :] = x[rows_e, :] @ W[e].T        W[e]: [1408, 2048]

Strategy: pure expert-parallel over 8 NeuronCores — core c owns experts
[8c, 8c+8) and their (contiguous) token rows. No collectives; gather is a
host-side concat. Host pre-transposes both operands into K-major fp16
layouts the PE consumes directly:
  x: [P=128 (contraction chunk), MT, KO=16, 128]  — m-tile blocks, so a
     chunk DMA is 128 fat (8 KB) lines and LDWEIGHTS reads stay dense.
  W: [E_loc, P, KO, OUT] — quarter-K loads span the full OUT row (2.75 KB
     lines), shared by all three N tiles.
Inner loop is n-innermost: one LDWEIGHTS of x per (m-tile, ko), three
PSUM banks accumulate the three N tiles in parallel; output is staged to
a full-width SBUF tile and written as whole 5.6 KB DRAM rows.
"""

import math

import numpy as np

import concourse.mybir as mybir
import concourse.tile as tile
from concourse import bacc
from concourse.bass_utils import run_bass_kernel_spmd

N_CORES = 8
P = 128
IN_SIZE = 2048
OUT_SIZE = 1408
KO = IN_SIZE // P  # 16 contraction subtiles
N_TILE = 512
BLK = 2  # m-tiles per x chunk DMA

# "f16" halves DRAM traffic vs fp32 and runs the PE single-pass at full
# rate; combined rel err ~2.5e-4.
W_DTYPE = "f16"
X_DTYPE = "f16"

_nc_cache: dict = {}


def _n_tiles():
    tiles = []
    n0 = 0
    while n0 < OUT_SIZE:
        nsz = min(N_TILE, OUT_SIZE - n0)
        tiles.append((n0, nsz))
        n0 += nsz
    return tiles


SEG_MAX = 6 * P  # per-segment token cap: bounds SBUF for arbitrary splits
KQ = 4  # W arrives in quarter-K chunks for fine pipelining


def _build(pattern: tuple) -> "bacc.Bacc":
    """One SPMD program: `pattern` = per-expert (padded) token counts for the
    8 local experts of a core; identical across cores. Experts larger than
    SEG_MAX are processed in segments (W reloaded per segment)."""
    T = sum(pattern)
    E_loc = len(pattern)
    MT = T // P
    nc = bacc.Bacc(None, target_bir_lowering=False, name="grouped_linear")
    x_dt = mybir.dt.float16 if X_DTYPE == "f16" else mybir.dt.float32r
    w_dt = mybir.dt.float16 if W_DTYPE == "f16" else mybir.dt.float32r
    xT = nc.dram_tensor("xT", [P, MT, KO, P], x_dt, kind="ExternalInput")
    wT = nc.dram_tensor(
        "wT", [E_loc, P, KO, OUT_SIZE], w_dt, kind="ExternalInput"
    )
    y = nc.dram_tensor("y", [T, OUT_SIZE], mybir.dt.float32, kind="ExternalOutput")

    # segments: (expert, token offset, token count<=SEG_MAX)
    segs = []
    t = 0
    for e in range(E_loc):
        m = pattern[e]
        s0 = 0
        while s0 < m:
            sm = min(SEG_MAX, m - s0)
            segs.append((e, t + s0, sm))
            s0 += sm
        t += m
    # Pair ordering (tuned for the alternating small/big pattern): larger
    # segment of each pair first so the small one rides in its DMA shadow.
    order = []
    for i in range(0, len(segs) - 1, 2):
        a, b = i, i + 1
        order.extend([b, a] if segs[b][2] > segs[a][2] else [a, b])
    if len(segs) % 2:
        order.append(len(segs) - 1)

    n_tiles = _n_tiles()

    # HAM warm-up schedule: N=128 body + fine N=64 tail, sized to keep the
    # PE busy from engine start (~7.5 us) through first-data (~12.4 us)
    # with no idle gap (idle re-throttles the clock), while keeping the
    # cost of overshooting data arrival to tens of ns per excess matmul.
    WARM_NS = [128] * 32 + [64] * 40

    with tile.TileContext(nc) as tc:
        with (
            tc.tile_pool(name="xp", bufs=5) as xpool,
            tc.tile_pool(name="wp", bufs=9) as wpool,
            tc.tile_pool(name="op", bufs=3) as opool,
            tc.tile_pool(name="ps", bufs=7, space="PSUM") as pspool,
            tc.tile_pool(name="wu", bufs=1) as wupool,
            tc.tile_pool(name="wups", bufs=1, space="PSUM") as wupspool,
        ):
            # Warm-up: keep the PE's HAM activity window busy while the
            # first x/W DMAs are in flight, so the real matmul stream
            # starts at the full (warm) clock instead of 1.2 GHz.
            wx = wupool.tile([P, P], x_dt, tag="wx", name="wx")
            nc.vector.memset(wx[:, :], 0.0)
            wps = wupspool.tile([P, P], mybir.dt.float32, tag="wps", name="wps")
            for i, wn in enumerate(WARM_NS):
                nc.tensor.matmul(
                    wps[:, :wn],
                    wx[:, :],
                    wx[:, :wn],
                    start=(i == 0),
                    stop=(i == len(WARM_NS) - 1),
                )

            for oi, si in enumerate(order):
                e, t0, m = segs[si]
                mts = m // P
                mt0 = t0 // P
                # W ko-chunk split: first segment leads with a single-ko
                # chunk so the first matmul only waits on ~1.4 MB of DMA.
                if oi == 0:
                    ksplit = [1, 1, 2] + [KQ] * ((KO - KQ) // KQ)
                else:
                    ksplit = [KQ] * (KO // KQ)

                w_chunks = []  # (ko0, kn, tile)
                x_cs = []

                def _load_w(ci, ko0, kn):
                    w_q = wpool.tile([P, KQ, OUT_SIZE], w_dt, tag="w", name="w_q")
                    nc.sync.dma_start(
                        w_q[:, :kn, :], wT[e, :, ko0 : ko0 + kn, :]
                    )
                    w_chunks.append((ko0, kn, w_q))

                xmap = []  # m-tile -> (chunk idx, block idx)

                def _load_x(c0, blk, split=False):
                    bsz = min(blk, mts - c0)
                    x_c = xpool.tile([P, BLK, KO, P], x_dt, tag="x", name="x_c")
                    if split:
                        # per-m-tile DMAs so the first matmul only waits on
                        # the first block (if Tile tracks subregions)
                        for b in range(bsz):
                            nc.sync.dma_start(
                                x_c[:, b : b + 1, :, :],
                                xT[:, mt0 + c0 + b : mt0 + c0 + b + 1, :, :],
                            )
                    else:
                        nc.sync.dma_start(
                            x_c[:, :bsz, :, :], xT[:, mt0 + c0 : mt0 + c0 + bsz, :, :]
                        )
                    ci = len(x_cs)
                    x_cs.append(x_c)
                    xmap.extend((ci, b) for b in range(bsz))

                # Issue order: first W chunk, first x chunk, rest of W,
                # rest of x — keeps the first matmul's critical path short.
                ko0 = 0
                _load_w(0, ko0, ksplit[0])
                ko0 += ksplit[0]
                _load_x(0, BLK, split=(oi == 0))
                for ci, kn in enumerate(ksplit[1:], start=1):
                    _load_w(ci, ko0, kn)
                    ko0 += kn
                while len(xmap) < mts:
                    _load_x(len(xmap), BLK)

                last_mt = oi == len(order) - 1

                def _finish(mt, pss):
                    o_t = opool.tile([P, OUT_SIZE], mybir.dt.float32, tag="o", name="o_t")
                    if last_mt and mt == mts - 1:
                        # Tail: store per n-tile so the first two stores
                        # overlap the last n-tile's matmuls/copy.
                        for ni, (n0, nsz) in enumerate(n_tiles):
                            nc.vector.tensor_copy(o_t[:, n0 : n0 + nsz], pss[ni][:, :nsz])
                            nc.scalar.dma_start(
                                y[t0 + mt * P : t0 + (mt + 1) * P, n0 : n0 + nsz],
                                o_t[:, n0 : n0 + nsz],
                            )
                        return
                    for ni, (n0, nsz) in enumerate(n_tiles):
                        nc.vector.tensor_copy(o_t[:, n0 : n0 + nsz], pss[ni][:, :nsz])
                    nc.scalar.dma_start(
                        y[t0 + mt * P : t0 + (mt + 1) * P, :], o_t
                    )

                mt_start = 0
                if oi == 0:
                    # First pair k-outer: consume W ko-chunks as they land
                    # so the PE isn't starved during the 8.6 MB bootstrap.
                    psz = min(2, mts)
                    pss2 = [
                        [
                            pspool.tile(
                                [P, N_TILE], mybir.dt.float32, tag="ps", name="ps_t"
                            )
                            for _ in n_tiles
                        ]
                        for _ in range(psz)
                    ]
                    for ko0, kn, w_q in w_chunks:
                        for k in range(kn):
                            ko = ko0 + k
                            for j in range(psz):
                                x_c = x_cs[xmap[j][0]]
                                b = xmap[j][1]
                                for ni, (n0, nsz) in enumerate(n_tiles):
                                    nc.tensor.matmul(
                                        pss2[j][ni][:, :nsz],
                                        x_c[:, b, ko, :],
                                        w_q[:, k, n0 : n0 + nsz],
                                        start=(ko == 0),
                                        stop=(ko == KO - 1),
                                    )
                    for j in range(psz):
                        _finish(j, pss2[j])
                    mt_start = psz

                for mt in range(mt_start, mts):
                    x_c = x_cs[xmap[mt][0]]
                    b = xmap[mt][1]
                    pss = [
                        pspool.tile([P, N_TILE], mybir.dt.float32, tag="ps", name="ps_t")
                        for _ in n_tiles
                    ]
                    for ko0, kn, w_q in w_chunks:
                        for k in range(kn):
                            ko = ko0 + k
                            for ni, (n0, nsz) in enumerate(n_tiles):
                                nc.tensor.matmul(
                                    pss[ni][:, :nsz],
                                    x_c[:, b, ko, :],
                                    w_q[:, k, n0 : n0 + nsz],
                                    start=(ko == 0),
                                    stop=(ko == KO - 1),
                                )
                    _finish(mt, pss)
    nc.compile()
    return nc


def _get_nc(pattern: tuple) -> "bacc.Bacc":
    nc = _nc_cache.get((pattern, W_DTYPE, X_DTYPE))
    if nc is None:
        nc = _build(pattern)
        _nc_cache[(pattern, W_DTYPE, X_DTYPE)] = nc
    return nc


def _plan(splits: np.ndarray):
    """Choose a per-core expert-size pattern (identical across cores, sizes
    multiples of 128). Returns (padded_pattern, per-core list of per-expert
    actual sizes)."""
    E = len(splits)
    epc = E // N_CORES
    per_core = [tuple(int(s) for s in splits[c * epc : (c + 1) * epc]) for c in range(N_CORES)]
    uniform = all(p == per_core[0] for p in per_core)
    if uniform:
        padded = tuple(128 * math.ceil(s / 128) for s in per_core[0])
    else:
        m_pad = 128 * math.ceil(int(max(splits.max(), 1)) / 128)
        padded = (m_pad,) * epc
    return padded, per_core


def kernel(x: np.ndarray, W: np.ndarray, m_splits: np.ndarray, _profile=None) -> np.ndarray:
    x = np.ascontiguousarray(np.asarray(x), dtype=np.float32)
    W = np.ascontiguousarray(np.asarray(W), dtype=np.float32)
    raw = np.asarray(m_splits).astype(np.int64)
    E = raw.shape[0]
    assert E % N_CORES == 0 and W.shape[0] == E
    epc = E // N_CORES
    # Mirror the reference's python-slice semantics: x[offs[e]:offs[e+1]]
    # clips to the array bounds, so effective sizes come from clipped offsets.
    raw_offs = np.concatenate([[0], np.cumsum(np.maximum(raw, 0))])
    lo = np.minimum(raw_offs[:-1], x.shape[0])
    hi = np.minimum(raw_offs[1:], x.shape[0])
    splits = np.maximum(hi - lo, 0)
    offs = np.concatenate([[0], np.cumsum(splits)])
    total = int(offs[-1])

    padded, per_core = _plan(splits)
    pofs = np.concatenate([[0], np.cumsum(padded)])
    T_pad = int(pofs[-1])

    nc = _get_nc(padded)

    in_maps = []
    for c in range(N_CORES):
        if tuple(padded) == per_core[c]:
            xs = x[lo[c * epc] : hi[(c + 1) * epc - 1]]
        else:
            xs = np.zeros((T_pad, IN_SIZE), dtype=np.float32)
            for e in range(epc):
                g = c * epc + e
                xs[pofs[e] : pofs[e] + splits[g]] = x[lo[g] : hi[g]]
        # [T, IN] -> [P, MT, KO, 128]: m-tile-blocked, contraction on partitions
        xTc = xs.reshape(T_pad // P, P, KO, P).transpose(3, 0, 2, 1)
        if X_DTYPE == "f16":
            xTc = xTc.astype(np.float16)
        xT = np.ascontiguousarray(xTc)
        wTc = W[c * epc : (c + 1) * epc].reshape(epc, OUT_SIZE, KO, P).transpose(0, 3, 2, 1)
        if W_DTYPE == "f16":
            wTc = wTc.astype(np.float16)
        wT = np.ascontiguousarray(wTc)
        in_maps.append({"xT": xT, "wT": wT})

    kwargs = dict(_profile) if _profile else {}
    res = run_bass_kernel_spmd(nc, in_maps, core_ids=list(range(N_CORES)), **kwargs)
    if _profile is not None:
        _profile["result"] = res

    out = np.empty((total, OUT_SIZE), dtype=np.float32)
    for c in range(N_CORES):
        yc = res.results[c]["y"]
        for e in range(epc):
            g = c * epc + e
            out[offs[g] : offs[g + 1]] = yc[pofs[e] : pofs[e] + splits[g]]
    return out
